# revision 9
# baseline (speedup 1.0000x reference)
"""Self-contained Trainium2 Bass kernel for the 2-layer GCN problem.

kernel(**inputs) takes the FULL inputs (x [50000,128] f32, edge_index [2,600000] i32,
W1,b1,gamma,beta,Wf,bf,Wo,bo) and returns (features [50000,128], logits [50000,64]).

Strategy (8 NeuronCores, SPMD):
  - 1D node partition: core c owns nodes [c*6250, (c+1)*6250), padded to 6272 = 49x128.
  - Edges partitioned by destination core; per core, bucketed by (dst tile of 128,
    src half) with a uniform bucket capacity so a single SPMD program serves all cores.
  - GCN layer as agg = dinv[dst] * sum_{e->dst} (dinv[src]*x_src): the src-side scale
    is pre-applied to the gathered feature rows (xs on host, hs on device), the
    dst-side scale is a per-partition scalar after the dense transform.
  - Source rows fetched with dma_gather (bf16, int16 indices -> two 25088-row halves,
    1024 idxs/call spread over 4 SWDGE queues); the segment-sum is a PE matmul
    (bf16, fp32 PSUM accumulation) against a host-precomputed one-hot selector
    streamed from DRAM (identical for both layers).
  - dinv computed on device by a small single-core program from CSR row pointers
    (sort metadata); the host lays out the per-edge tables with it.
  - BatchNorm stats via mask-vector matmuls accumulated in PSUM + AllReduce;
    dense transforms and BN in fp32. Hidden features AllGathered in bf16.
"""
import math
import os
import sys
from contextlib import ExitStack

import numpy as np

sys.path.insert(0, "/opt/trn_rl_repo")
sys.path.insert(0, "/opt/pypackages")

import ml_dtypes
import concourse.bacc as bacc
import concourse.bass as bass
import concourse.tile as tile
from concourse import mybir
from concourse.bass_utils import run_bass_kernel_spmd

F32 = mybir.dt.float32
BF16 = mybir.dt.bfloat16
I16 = mybir.dt.int16
NP_BF16 = ml_dtypes.bfloat16

CFG = dict(N=50000, E=600000, C=128, HID=128, OUT=64, M=8, GCALL=1024, EPS=1e-5,
           SB=64, NQ=4)

LAST_EXEC_NS = None
LAST_TRACE = None
LAST_PROFILE_JSON = None


def _ensure_ntff_hook():
    """Register a minimal antenv.axon_hooks shim (NTFF profiling via the
    axon .so) when the full module isn't present in this image."""
    import types
    try:
        from antenv.axon_hooks import get_axon_ntff_profile_hook  # noqa: F401
        return True
    except ImportError:
        pass
    try:
        from trn_agent_boot.trn_boot import _ntff_profile_via_ctypes
        hook = _ntff_profile_via_ctypes("/opt/axon/libaxon_pjrt.so")
        if hook is None:
            return False
        import antenv
        mod = types.ModuleType("antenv.axon_hooks")
        _state = {"hook": hook}
        mod.get_axon_ntff_profile_hook = lambda: _state["hook"]
        mod.set_axon_ntff_profile_hook = lambda h: _state.__setitem__("hook", h)
        antenv.axon_hooks = mod
        sys.modules["antenv.axon_hooks"] = mod
        return True
    except Exception:
        return False


def _derive(cfg):
    d = dict(cfg)
    N, M = cfg["N"], cfg["M"]
    nper = (N + M - 1) // M
    TPC = (nper + 127) // 128
    npc = TPC * 128
    npad = M * npc
    half = npad // 2
    assert half <= 32768, "int16 gather index limit"
    assert M % 2 == 0
    d.update(nper=nper, TPC=TPC, npc=npc, npad=npad, half=half)
    return d


# ---------------------------------------------------------------------------
# Host-side layout prep
# ---------------------------------------------------------------------------

def prep_graph(edge_index, cfg):
    """Bucket edges by (dst core, dst tile, src half). Pure layout metadata."""
    c = _derive(cfg)
    N, M, nper, TPC, npc, half = c["N"], c["M"], c["nper"], c["TPC"], c["npc"], c["half"]
    src = np.asarray(edge_index[0], dtype=np.int64)
    dst = np.asarray(edge_index[1], dtype=np.int64)
    core = dst // nper
    srcp = (src // nper) * npc + (src % nper)  # padded global src id

    per_core = []
    maxb = 0
    for ci in range(M):
        m = core == ci
        dl = dst[m] - ci * nper
        sp = srcp[m]
        hsel = (sp >= half).astype(np.int64)
        t = dl // 128
        o = dl % 128
        key = t * 2 + hsel
        order = np.argsort(key, kind="stable")
        key_s, sp_s, o_s = key[order], sp[order], o[order]
        counts = np.bincount(key_s, minlength=TPC * 2)
        maxb = max(maxb, int(counts.max()) if counts.size else 0)
        dl_sorted = np.sort(dl)
        pts = np.arange(npc)
        rlo = np.searchsorted(dl_sorted, pts, side="left").astype(np.float32)
        rhi = np.searchsorted(dl_sorted, pts, side="right").astype(np.float32)
        per_core.append(dict(sp_s=sp_s, o_s=o_s, counts=counts, rlo=rlo, rhi=rhi))
    capH = max(((maxb + 127) // 128) * 128, 128)
    LH = TPC * capH
    GCALL = cfg["GCALL"]
    LHP = ((LH + GCALL - 1) // GCALL) * GCALL
    c.update(capH=capH, LH=LH, LHP=LHP)
    return c, per_core


def prep_tables(c, per_core, dinv_pad):
    """Per-core device tables. dinv_pad: [npad] f32 (device-computed)."""
    M, TPC, capH, LH, LHP, half, npc, nper = (c["M"], c["TPC"], c["capH"], c["LH"],
                                              c["LHP"], c["half"], c["npc"], c["nper"])
    RCH = LH // 128
    out = []
    for ci in range(M):
        pc = per_core[ci]
        src_stream = np.zeros((2, LHP), np.int64)
        dstv = np.full((2, LH), -1.0, np.float32)
        starts = np.zeros(TPC * 2 + 1, np.int64)
        np.cumsum(pc["counts"], out=starts[1:])
        for t in range(TPC):
            for h in (0, 1):
                k = t * 2 + h
                b0, b1 = starts[k], starts[k + 1]
                n_b = b1 - b0
                pos = t * capH
                src_stream[h, pos:pos + n_b] = pc["sp_s"][b0:b1] - h * half
                dstv[h, pos:pos + n_b] = pc["o_s"][b0:b1].astype(np.float32)
        idxs = []
        for h in (0, 1):
            w16 = src_stream[h].astype(np.int16).reshape(-1, 16).T
            idxs.append(np.tile(w16, (8, 1)).copy())
        # host-built one-hot selector, [128, 2*RCH*128] bf16, chunk-major
        dstv_w = np.concatenate([dstv[0].reshape(RCH, 128).T,
                                 dstv[1].reshape(RCH, 128).T], axis=1)  # [128, 2RCH]
        S = (dstv_w[:, :, None] == np.arange(128, dtype=np.float32)[None, None, :])
        S = S.astype(NP_BF16).reshape(128, 2 * RCH * 128)
        dloc = dinv_pad[ci * npc:(ci + 1) * npc].reshape(TPC, 128).T.copy()
        nv = min(nper, c["N"] - ci * nper)
        vmask = (np.arange(npc) < nv).astype(np.float32).reshape(TPC, 128).T.copy()
        out.append(dict(idxA=idxs[0], idxB=idxs[1], S=np.ascontiguousarray(S),
                        dloc=np.ascontiguousarray(dloc),
                        valid=np.ascontiguousarray(vmask)))
    return out


def pad_x(x, c):
    npad, npc, nper, N = c["npad"], c["npc"], c["nper"], c["N"]
    xp = np.zeros((npad, x.shape[1]), np.float32)
    for ci in range(c["M"]):
        nv = min(nper, N - ci * nper)
        xp[ci * npc:ci * npc + nv] = x[ci * nper:ci * nper + nv]
    return xp


# ---------------------------------------------------------------------------
# Program 1: dinv = (deg>0) / sqrt(max(deg,1)) from CSR row pointers
# ---------------------------------------------------------------------------

def build_dinv_program(c):
    TPCg = c["M"] * c["TPC"]
    nc = bacc.Bacc("TRN2", target_bir_lowering=False, debug=False, num_devices=1)
    rlo_t = nc.dram_tensor("rlo", [128, TPCg], F32, kind="ExternalInput")
    rhi_t = nc.dram_tensor("rhi", [128, TPCg], F32, kind="ExternalInput")
    dinv_t = nc.dram_tensor("dinv", [128, TPCg], F32, kind="ExternalOutput")
    with tile.TileContext(nc) as tc, ExitStack() as ctx:
        pool = ctx.enter_context(tc.tile_pool(name="p", bufs=1))
        rlo = pool.tile([128, TPCg], F32)
        nc.sync.dma_start(rlo[:], rlo_t[:])
        rhi = pool.tile([128, TPCg], F32)
        nc.sync.dma_start(rhi[:], rhi_t[:])
        deg = pool.tile([128, TPCg], F32)
        nc.vector.tensor_sub(deg[:], rhi[:], rlo[:])
        degm = pool.tile([128, TPCg], F32)
        nc.vector.tensor_scalar_max(degm[:], deg[:], 1.0)
        sq = pool.tile([128, TPCg], F32)
        nc.scalar.sqrt(sq[:], degm[:])
        rs = pool.tile([128, TPCg], F32)
        nc.vector.reciprocal(rs[:], sq[:])
        mask = pool.tile([128, TPCg], F32)
        nc.vector.tensor_scalar(mask[:], deg[:], 0.0, None, mybir.AluOpType.is_gt)
        dinv = pool.tile([128, TPCg], F32)
        nc.vector.tensor_mul(dinv[:], rs[:], mask[:])
        nc.sync.dma_start(dinv_t[:], dinv[:])
    nc.compile()
    return nc


# ---------------------------------------------------------------------------
# Program 2: the full GCN
# ---------------------------------------------------------------------------

def build_main_program(c):
    M, TPC, capH, LH, LHP, half, npc, npad = (c["M"], c["TPC"], c["capH"], c["LH"],
                                              c["LHP"], c["half"], c["npc"], c["npad"])
    C, HID, OUT, GCALL, EPS, N = c["C"], c["HID"], c["OUT"], c["GCALL"], c["EPS"], c["N"]
    SB, NQ = c["SB"], c["NQ"]
    CPH = LHP // 128          # chunks per half (incl pad chunks)
    RCH = LH // 128           # real chunks per half
    CPT = capH // 128         # chunks per (tile, half) group
    NCALL = LHP // GCALL
    CPC = GCALL // 128        # chunks per gather call
    NSB = (2 * RCH + SB - 1) // SB   # S batches

    nc = bacc.Bacc("TRN2", target_bir_lowering=False, debug=False, num_devices=M,
                   num_swdge_queues=NQ)
    xp_t = nc.dram_tensor("x_pad", [npad, C], BF16, kind="ExternalInput")
    idxA_t = nc.dram_tensor("idxA", [128, LHP // 16], I16, kind="ExternalInput")
    idxB_t = nc.dram_tensor("idxB", [128, LHP // 16], I16, kind="ExternalInput")
    S_t = nc.dram_tensor("S", [128, 2 * RCH * 128], BF16, kind="ExternalInput")
    dloc_t = nc.dram_tensor("dloc", [128, TPC], F32, kind="ExternalInput")
    valid_t = nc.dram_tensor("valid", [128, TPC], F32, kind="ExternalInput")
    W1_t = nc.dram_tensor("W1", [C, HID], F32, kind="ExternalInput")
    Wf_t = nc.dram_tensor("Wf", [HID, C], F32, kind="ExternalInput")
    Wo_t = nc.dram_tensor("Wo", [HID, OUT], F32, kind="ExternalInput")
    gamma_t = nc.dram_tensor("gamma", [1, HID], F32, kind="ExternalInput")
    beta_t = nc.dram_tensor("beta", [1, HID], F32, kind="ExternalInput")
    bfb_t = nc.dram_tensor("bfb", [128, C], F32, kind="ExternalInput")
    bob_t = nc.dram_tensor("bob", [128, OUT], F32, kind="ExternalInput")
    feat_t = nc.dram_tensor("features", [npc, C], F32, kind="ExternalOutput")
    logi_t = nc.dram_tensor("logits", [npc, OUT], F32, kind="ExternalOutput")

    groups = [list(range(M))]

    with tile.TileContext(nc) as tc, ExitStack() as ctx:
        const = ctx.enter_context(tc.tile_pool(name="const", bufs=1))
        tab = ctx.enter_context(tc.tile_pool(name="tab", bufs=1))
        zpool = ctx.enter_context(tc.tile_pool(name="zpool", bufs=1))
        accp = ctx.enter_context(tc.tile_pool(name="accp", bufs=1))
        mpool = ctx.enter_context(tc.tile_pool(name="mpool", bufs=6))
        spool = ctx.enter_context(tc.tile_pool(name="spool", bufs=2))
        tmpp = ctx.enter_context(tc.tile_pool(name="tmpp", bufs=4))
        psA = ctx.enter_context(tc.tile_pool(name="psA", bufs=2, space="PSUM"))
        psY = ctx.enter_context(tc.tile_pool(name="psY", bufs=2, space="PSUM"))
        psS = ctx.enter_context(tc.tile_pool(name="psS", bufs=1, space="PSUM"))
        dram = ctx.enter_context(tc.tile_pool(name="dram", bufs=1, space="DRAM"))

        ones1 = const.tile([1, 128], F32)
        nc.gpsimd.memset(ones1[:], 1.0)
        W1_sb = const.tile([C, HID], F32)
        nc.sync.dma_start(W1_sb[:], W1_t[:])
        Wf_sb = const.tile([HID, C], F32)
        nc.sync.dma_start(Wf_sb[:], Wf_t[:])
        Wo_sb = const.tile([HID, OUT], F32)
        nc.sync.dma_start(Wo_sb[:], Wo_t[:])
        gamma_sb = const.tile([1, HID], F32)
        nc.sync.dma_start(gamma_sb[:], gamma_t[:])
        beta_sb = const.tile([1, HID], F32)
        nc.sync.dma_start(beta_sb[:], beta_t[:])
        bfb_sb = const.tile([128, C], F32)
        nc.sync.dma_start(bfb_sb[:], bfb_t[:])
        bob_sb = const.tile([128, OUT], F32)
        nc.sync.dma_start(bob_sb[:], bob_t[:])
        dloc_sb = const.tile([128, TPC], F32)
        nc.sync.dma_start(dloc_sb[:], dloc_t[:])
        valid_sb = const.tile([128, TPC], F32)
        nc.sync.dma_start(valid_sb[:], valid_t[:])

        idx_sb = []
        for nm, t in (("idxA", idxA_t), ("idxB", idxB_t)):
            s = tab.tile([128, LHP // 16], I16, name=nm + "_sb", tag=nm)
            nc.sync.dma_start(s[:], t[:])
            idx_sb.append(s)

        h_loc = dram.tile([npc, C], BF16)
        h_full = dram.tile([npad, C], BF16, addr_space="Shared")
        st_in = dram.tile([2, HID], F32)
        st_out = dram.tile([2, HID], F32, addr_space="Shared")

        qctr = [0]

        def sparse_pass(src_dram, acc_prefix):
            """Gather + one-hot segment-sum both halves; returns dict t -> acc
            SBUF tile [C, 128] f32 (channels on partitions)."""
            accs = {}
            qi = 0            # real-chunk counter (indexes S batches)
            sb_tile = [None]
            for h in (0, 1):
                base = src_dram[h * half:(h + 1) * half, :]
                ps = None
                for k in range(NCALL):
                    msg = mpool.tile([128, CPC, C], BF16, name=f"msg_{acc_prefix}",
                                     tag="msg")
                    nc.gpsimd.dma_gather(msg[:], base,
                                         idx_sb[h][:, k * (GCALL // 16):(k + 1) * (GCALL // 16)],
                                         GCALL, GCALL, C, queue_num=qctr[0] % NQ)
                    qctr[0] += 1
                    for j in range(CPC):
                        qq = k * CPC + j
                        if qq >= RCH:
                            continue
                        if qi % SB == 0:
                            nb = min(SB, 2 * RCH - qi)
                            st = spool.tile([128, nb, 128], BF16,
                                            name=f"Sb_{acc_prefix}", tag="Sb")
                            nc.sync.dma_start(st[:], S_t[:, qi * 128:(qi + nb) * 128])
                            sb_tile[0] = st
                        t = qq // CPT
                        jj = qq % CPT
                        if jj == 0:
                            ps = psA.tile([C, 128], F32, name=f"ps_{acc_prefix}",
                                          tag="psacc")
                        nc.tensor.matmul(ps[:], lhsT=msg[:, j, :],
                                         rhs=sb_tile[0][:, qi % SB, :],
                                         start=(jj == 0), stop=(jj == CPT - 1))
                        qi += 1
                        if jj == CPT - 1:
                            if h == 0:
                                a = accp.tile([C, 128], F32, name=f"{acc_prefix}_{t}",
                                              tag=f"acc_{t}")
                                nc.vector.tensor_copy(a[:], ps[:])
                                accs[t] = a
                            else:
                                nc.vector.tensor_add(accs[t][:], accs[t][:], ps[:])
            return accs

        # ================= layer 1 =================
        accs1 = sparse_pass(xp_t, "l1")
        st_sum = psS.tile([1, HID], F32, tag="ssum")
        st_sq = psS.tile([1, HID], F32, tag="ssq")
        z_tiles = []
        for t in range(TPC):
            y = psY.tile([128, HID], F32, name="y1", tag="y")
            nc.tensor.matmul(y[:], lhsT=accs1[t][:], rhs=W1_sb[:], start=True, stop=True)
            z = zpool.tile([128, HID], F32, name=f"z_{t}", tag=f"z_{t}")
            nc.scalar.activation(z[:], y[:], mybir.ActivationFunctionType.Copy,
                                 scale=dloc_sb[:, t:t + 1])
            z_tiles.append(z)
            sq = tmpp.tile([128, HID], F32, name="sqz", tag="sqz")
            nc.scalar.square(sq[:], z[:])
            nc.tensor.matmul(st_sum[:], lhsT=valid_sb[:, t:t + 1], rhs=z[:],
                             start=(t == 0), stop=(t == TPC - 1))
            nc.tensor.matmul(st_sq[:], lhsT=valid_sb[:, t:t + 1], rhs=sq[:],
                             start=(t == 0), stop=(t == TPC - 1))

        # ================= BN stats -> coefficients =================
        st_sb0 = tmpp.tile([1, HID], F32, tag="stats0", bufs=1)
        nc.vector.tensor_copy(st_sb0[:], st_sum[:])
        st_sb1 = tmpp.tile([1, HID], F32, tag="stats1", bufs=1)
        nc.vector.tensor_copy(st_sb1[:], st_sq[:])
        nc.sync.dma_start(st_in[0:1, :], st_sb0[:])
        nc.sync.dma_start(st_in[1:2, :], st_sb1[:])
        nc.gpsimd.collective_compute("AllReduce", mybir.AluOpType.add,
                                     replica_groups=groups,
                                     ins=[st_in[:]], outs=[st_out[:]])
        st20 = tmpp.tile([1, HID], F32, tag="stats20", bufs=1)
        nc.sync.dma_start(st20[:], st_out[0:1, :])
        st21 = tmpp.tile([1, HID], F32, tag="stats21", bufs=1)
        nc.sync.dma_start(st21[:], st_out[1:2, :])
        invN = 1.0 / float(N)
        mean = tmpp.tile([1, HID], F32, tag="bnrow", bufs=8)
        nc.vector.tensor_scalar(mean[:], st20[:], invN, None, mybir.AluOpType.mult)
        ex2 = tmpp.tile([1, HID], F32, tag="bnrow", bufs=8)
        nc.vector.tensor_scalar(ex2[:], st21[:], invN, None, mybir.AluOpType.mult)
        msq = tmpp.tile([1, HID], F32, tag="bnrow", bufs=8)
        nc.scalar.square(msq[:], mean[:])
        var = tmpp.tile([1, HID], F32, tag="bnrow", bufs=8)
        nc.vector.tensor_sub(var[:], ex2[:], msq[:])
        varp = tmpp.tile([1, HID], F32, tag="bnrow", bufs=8)
        nc.vector.tensor_scalar_add(varp[:], var[:], float(EPS))
        std = tmpp.tile([1, HID], F32, tag="bnrow", bufs=8)
        nc.scalar.sqrt(std[:], varp[:])
        istd = tmpp.tile([1, HID], F32, tag="bnrow", bufs=8)
        nc.vector.reciprocal(istd[:], std[:])
        scal = tmpp.tile([1, HID], F32, tag="bnrow", bufs=8)
        nc.vector.tensor_mul(scal[:], gamma_sb[:], istd[:])
        mscal = tmpp.tile([1, HID], F32, tag="bnrow", bufs=8)
        nc.vector.tensor_mul(mscal[:], mean[:], scal[:])
        shif = tmpp.tile([1, HID], F32, tag="bnrow", bufs=8)
        nc.vector.tensor_sub(shif[:], beta_sb[:], mscal[:])
        scale_b = const.tile([128, HID], F32)
        shift_b = const.tile([128, HID], F32)
        for row, dst_tile in ((scal, scale_b), (shif, shift_b)):
            pb = psY.tile([128, HID], F32, name="pb", tag="y")
            nc.tensor.matmul(pb[:], lhsT=ones1[:], rhs=row[:], start=True, stop=True)
            nc.vector.tensor_copy(dst_tile[:], pb[:])

        # ========== BN apply + ReLU + dinv scale (hs, bf16) + AllGather ==========
        for t in range(TPC):
            t1 = tmpp.tile([128, HID], F32, name="bn1", tag="bn1")
            nc.vector.tensor_mul(t1[:], z_tiles[t][:], scale_b[:])
            t2 = tmpp.tile([128, HID], F32, name="bn2", tag="bn2")
            nc.vector.tensor_add(t2[:], t1[:], shift_b[:])
            ht = tmpp.tile([128, HID], BF16, name="ht", tag="ht")
            # relu(x)*d == relu(x*d) for d >= 0; dloc >= 0
            nc.scalar.activation(ht[:], t2[:], mybir.ActivationFunctionType.Relu,
                                 scale=dloc_sb[:, t:t + 1])
            nc.sync.dma_start(h_loc[t * 128:(t + 1) * 128, :], ht[:])
        nc.gpsimd.collective_compute("AllGather", mybir.AluOpType.bypass,
                                     replica_groups=groups,
                                     ins=[h_loc[:]], outs=[h_full[:]])

        # ================= layer 2 =================
        accs2 = sparse_pass(h_full, "l2")
        for t in range(TPC):
            yf = psY.tile([128, C], F32, name="yf", tag="y")
            nc.tensor.matmul(yf[:], lhsT=accs2[t][:], rhs=Wf_sb[:], start=True, stop=True)
            ft = tmpp.tile([128, C], F32, name="ft", tag="ft")
            nc.scalar.activation(ft[:], yf[:], mybir.ActivationFunctionType.Copy,
                                 scale=dloc_sb[:, t:t + 1])
            ft2 = tmpp.tile([128, C], F32, name="ft2", tag="ft2")
            nc.vector.tensor_add(ft2[:], ft[:], bfb_sb[:])
            nc.sync.dma_start(feat_t[t * 128:(t + 1) * 128, :], ft2[:])
            yl = psY.tile([128, OUT], F32, name="yl", tag="yl")
            nc.tensor.matmul(yl[:], lhsT=accs2[t][:], rhs=Wo_sb[:], start=True, stop=True)
            lt = tmpp.tile([128, OUT], F32, name="lt", tag="lt")
            nc.scalar.activation(lt[:], yl[:], mybir.ActivationFunctionType.Copy,
                                 scale=dloc_sb[:, t:t + 1])
            lt2 = tmpp.tile([128, OUT], F32, name="lt2", tag="lt2")
            nc.vector.tensor_add(lt2[:], lt[:], bob_sb[:])
            nc.sync.dma_start(logi_t[t * 128:(t + 1) * 128, :], lt2[:])

    nc.compile()
    return nc


# ---------------------------------------------------------------------------
# kernel()
# ---------------------------------------------------------------------------

def run_gcn(x, edge_index, W1, b1, gamma, beta, Wf, bf, Wo, bo, cfg=None,
            runner=None, trace=False):
    """Full pipeline. runner: callable(nc, in_maps, core_ids) -> results list
    (defaults to hardware via run_bass_kernel_spmd)."""
    global LAST_EXEC_NS, LAST_TRACE, LAST_PROFILE_JSON
    cfg = dict(CFG if cfg is None else cfg)
    c, per_core = prep_graph(edge_index, cfg)
    M, TPC, npc, nper, N = c["M"], c["TPC"], c["npc"], c["nper"], c["N"]

    x = np.asarray(x, np.float32)
    W1 = np.asarray(W1, np.float32)
    gamma = np.asarray(gamma, np.float32).reshape(1, -1)
    beta = np.asarray(beta, np.float32).reshape(1, -1)
    Wf = np.asarray(Wf, np.float32)
    bf = np.asarray(bf, np.float32)
    Wo = np.asarray(Wo, np.float32)
    bo = np.asarray(bo, np.float32)

    # --- program 1: dinv on device (single core) ---
    rlo = np.concatenate([pc["rlo"].reshape(TPC, 128).T for pc in per_core], axis=1)
    rhi = np.concatenate([pc["rhi"].reshape(TPC, 128).T for pc in per_core], axis=1)
    nc1 = build_dinv_program(c)
    in1 = [dict(rlo=np.ascontiguousarray(rlo), rhi=np.ascontiguousarray(rhi))]
    if runner is None:
        res1 = run_bass_kernel_spmd(nc1, in1, [0]).results
    else:
        res1 = runner(nc1, in1, [0])
    dinv_pad = res1[0]["dinv"].T.reshape(-1)         # padded-global order

    # --- host table prep (device-computed dinv, host layout) ---
    tables = prep_tables(c, per_core, dinv_pad)
    xp = pad_x(x, c)
    xs = (xp * dinv_pad[:, None]).astype(NP_BF16)    # src-side scale folded in
    bfb = np.tile(bf.reshape(1, -1), (128, 1)).astype(np.float32)
    bob = np.tile(bo.reshape(1, -1), (128, 1)).astype(np.float32)

    # --- program 2 ---
    nc2 = build_main_program(c)
    in_maps = []
    for ci in range(M):
        tb = tables[ci]
        in_maps.append(dict(x_pad=xs, idxA=tb["idxA"], idxB=tb["idxB"], S=tb["S"],
                            dloc=tb["dloc"], valid=tb["valid"], W1=W1, Wf=Wf, Wo=Wo,
                            gamma=gamma, beta=beta, bfb=bfb, bob=bob))
    core_ids = list(range(M))
    if runner is None:
        if trace and not _ensure_ntff_hook():
            trace = False
        if trace:
            import concourse.bass_utils as _bu
            _bu.upload_artifacts = lambda d: str(d)
        try:
            r = run_bass_kernel_spmd(nc2, in_maps, core_ids, trace=trace)
        except Exception:
            if not trace:
                raise
            import traceback
            traceback.print_exc()
            print("trace path failed; re-running without trace", file=sys.stderr)
            r = run_bass_kernel_spmd(nc2, in_maps, core_ids, trace=False)
        if getattr(r, "exec_time_ns", None):
            LAST_EXEC_NS = r.exec_time_ns
        if getattr(r, "instructions_and_trace", None):
            LAST_TRACE = r.instructions_and_trace[1]
        if getattr(r, "profile_json", None):
            LAST_PROFILE_JSON = r.profile_json
        res2 = r.results
    else:
        res2 = runner(nc2, in_maps, core_ids)

    features = np.zeros((N, c["C"]), np.float32)
    logits = np.zeros((N, c["OUT"]), np.float32)
    for ci in range(M):
        nv = min(nper, N - ci * nper)
        features[ci * nper:ci * nper + nv] = res2[ci]["features"][:nv]
        logits[ci * nper:ci * nper + nv] = res2[ci]["logits"][:nv]
    return features, logits


def kernel(**inputs):
    feats, logits = run_gcn(**inputs)
    return feats, logits


if __name__ == "__main__":
    pass


# revision 10
# speedup vs baseline: 1.0094x; 1.0094x over previous
"""Self-contained Trainium2 Bass kernel for the 2-layer GCN problem.

kernel(**inputs) takes the FULL inputs (x [50000,128] f32, edge_index [2,600000] i32,
W1,b1,gamma,beta,Wf,bf,Wo,bo) and returns (features [50000,128], logits [50000,64]).

Strategy (8 NeuronCores, SPMD):
  - 1D node partition: core c owns nodes [c*6250, (c+1)*6250), padded to 6272 = 49x128.
  - Edges partitioned by destination core; per core, bucketed by (dst tile of 128,
    src half) with a uniform bucket capacity so a single SPMD program serves all cores.
  - GCN layer as agg = dinv[dst] * sum_{e->dst} (dinv[src]*x_src): the src-side scale
    is pre-applied to the gathered feature rows (xs on host, hs on device), the
    dst-side scale is a per-partition scalar after the dense transform.
  - Source rows fetched with dma_gather (bf16, int16 indices -> two 25088-row halves,
    1024 idxs/call spread over 4 SWDGE queues); the segment-sum is a PE matmul
    (bf16, fp32 PSUM accumulation) against a host-precomputed one-hot selector
    streamed from DRAM (identical for both layers).
  - dinv computed on device by a small single-core program from CSR row pointers
    (sort metadata); the host lays out the per-edge tables with it.
  - BatchNorm stats via mask-vector matmuls accumulated in PSUM + AllReduce;
    dense transforms and BN in fp32. Hidden features AllGathered in bf16.
"""
import math
import os
import sys
from contextlib import ExitStack

import numpy as np

sys.path.insert(0, "/opt/trn_rl_repo")
sys.path.insert(0, "/opt/pypackages")

import ml_dtypes
import concourse.bacc as bacc
import concourse.bass as bass
import concourse.tile as tile
from concourse import mybir
from concourse.bass_utils import run_bass_kernel_spmd

F32 = mybir.dt.float32
BF16 = mybir.dt.bfloat16
I16 = mybir.dt.int16
NP_BF16 = ml_dtypes.bfloat16

CFG = dict(N=50000, E=600000, C=128, HID=128, OUT=64, M=8, GCALL=1024, EPS=1e-5,
           SB=64, NQ=4, SCRATCH=32768, MBUFS=12)

LAST_EXEC_NS = None
LAST_TRACE = None
LAST_PROFILE_JSON = None


def _ensure_ntff_hook():
    """Register a minimal antenv.axon_hooks shim (NTFF profiling via the
    axon .so) when the full module isn't present in this image."""
    import types
    try:
        from antenv.axon_hooks import get_axon_ntff_profile_hook  # noqa: F401
        return True
    except ImportError:
        pass
    try:
        from trn_agent_boot.trn_boot import _ntff_profile_via_ctypes
        hook = _ntff_profile_via_ctypes("/opt/axon/libaxon_pjrt.so")
        if hook is None:
            return False
        import antenv
        mod = types.ModuleType("antenv.axon_hooks")
        _state = {"hook": hook}
        mod.get_axon_ntff_profile_hook = lambda: _state["hook"]
        mod.set_axon_ntff_profile_hook = lambda h: _state.__setitem__("hook", h)
        antenv.axon_hooks = mod
        sys.modules["antenv.axon_hooks"] = mod
        return True
    except Exception:
        return False


def _derive(cfg):
    d = dict(cfg)
    N, M = cfg["N"], cfg["M"]
    nper = (N + M - 1) // M
    TPC = (nper + 127) // 128
    npc = TPC * 128
    npad = M * npc
    half = npad // 2
    assert half <= 32768, "int16 gather index limit"
    assert M % 2 == 0
    d.update(nper=nper, TPC=TPC, npc=npc, npad=npad, half=half)
    return d


# ---------------------------------------------------------------------------
# Host-side layout prep
# ---------------------------------------------------------------------------

def prep_graph(edge_index, cfg):
    """Bucket edges by (dst core, dst tile, src half). Pure layout metadata."""
    c = _derive(cfg)
    N, M, nper, TPC, npc, half = c["N"], c["M"], c["nper"], c["TPC"], c["npc"], c["half"]
    src = np.asarray(edge_index[0], dtype=np.int64)
    dst = np.asarray(edge_index[1], dtype=np.int64)
    core = dst // nper
    srcp = (src // nper) * npc + (src % nper)  # padded global src id

    per_core = []
    maxb = 0
    for ci in range(M):
        m = core == ci
        dl = dst[m] - ci * nper
        sp = srcp[m]
        hsel = (sp >= half).astype(np.int64)
        t = dl // 128
        o = dl % 128
        key = t * 2 + hsel
        order = np.argsort(key, kind="stable")
        key_s, sp_s, o_s = key[order], sp[order], o[order]
        counts = np.bincount(key_s, minlength=TPC * 2)
        maxb = max(maxb, int(counts.max()) if counts.size else 0)
        dl_sorted = np.sort(dl)
        pts = np.arange(npc)
        rlo = np.searchsorted(dl_sorted, pts, side="left").astype(np.float32)
        rhi = np.searchsorted(dl_sorted, pts, side="right").astype(np.float32)
        per_core.append(dict(sp_s=sp_s, o_s=o_s, counts=counts, rlo=rlo, rhi=rhi))
    capH = max(((maxb + 127) // 128) * 128, 128)
    LH = TPC * capH
    GCALL = cfg["GCALL"]
    LHP = ((LH + GCALL - 1) // GCALL) * GCALL
    c.update(capH=capH, LH=LH, LHP=LHP)
    return c, per_core


def prep_tables(c, per_core, dinv_pad):
    """Per-core device tables. dinv_pad: [npad] f32 (device-computed)."""
    M, TPC, capH, LH, LHP, half, npc, nper = (c["M"], c["TPC"], c["capH"], c["LH"],
                                              c["LHP"], c["half"], c["npc"], c["nper"])
    RCH = LH // 128
    out = []
    for ci in range(M):
        pc = per_core[ci]
        src_stream = np.zeros((2, LHP), np.int64)
        dstv = np.full((2, LH), -1.0, np.float32)
        starts = np.zeros(TPC * 2 + 1, np.int64)
        np.cumsum(pc["counts"], out=starts[1:])
        for t in range(TPC):
            for h in (0, 1):
                k = t * 2 + h
                b0, b1 = starts[k], starts[k + 1]
                n_b = b1 - b0
                pos = t * capH
                src_stream[h, pos:pos + n_b] = pc["sp_s"][b0:b1] - h * half
                dstv[h, pos:pos + n_b] = pc["o_s"][b0:b1].astype(np.float32)
        idxs = []
        for h in (0, 1):
            w16 = src_stream[h].astype(np.int16).reshape(-1, 16).T
            idxs.append(np.tile(w16, (8, 1)).copy())
        # host-built one-hot selector, [128, 2*RCH*128] bf16, chunk-major
        dstv_w = np.concatenate([dstv[0].reshape(RCH, 128).T,
                                 dstv[1].reshape(RCH, 128).T], axis=1)  # [128, 2RCH]
        S = (dstv_w[:, :, None] == np.arange(128, dtype=np.float32)[None, None, :])
        S = S.astype(NP_BF16).reshape(128, 2 * RCH * 128)
        dloc = dinv_pad[ci * npc:(ci + 1) * npc].reshape(TPC, 128).T.copy()
        nv = min(nper, c["N"] - ci * nper)
        vmask = (np.arange(npc) < nv).astype(np.float32).reshape(TPC, 128).T.copy()
        out.append(dict(idxA=idxs[0], idxB=idxs[1], S=np.ascontiguousarray(S),
                        dloc=np.ascontiguousarray(dloc),
                        valid=np.ascontiguousarray(vmask)))
    return out


def pad_x(x, c):
    npad, npc, nper, N = c["npad"], c["npc"], c["nper"], c["N"]
    xp = np.zeros((npad, x.shape[1]), np.float32)
    for ci in range(c["M"]):
        nv = min(nper, N - ci * nper)
        xp[ci * npc:ci * npc + nv] = x[ci * nper:ci * nper + nv]
    return xp


# ---------------------------------------------------------------------------
# Program 1: dinv = (deg>0) / sqrt(max(deg,1)) from CSR row pointers
# ---------------------------------------------------------------------------

def build_dinv_program(c):
    TPCg = c["M"] * c["TPC"]
    nc = bacc.Bacc("TRN2", target_bir_lowering=False, debug=False, num_devices=1)
    rlo_t = nc.dram_tensor("rlo", [128, TPCg], F32, kind="ExternalInput")
    rhi_t = nc.dram_tensor("rhi", [128, TPCg], F32, kind="ExternalInput")
    dinv_t = nc.dram_tensor("dinv", [128, TPCg], F32, kind="ExternalOutput")
    with tile.TileContext(nc) as tc, ExitStack() as ctx:
        pool = ctx.enter_context(tc.tile_pool(name="p", bufs=1))
        rlo = pool.tile([128, TPCg], F32)
        nc.sync.dma_start(rlo[:], rlo_t[:])
        rhi = pool.tile([128, TPCg], F32)
        nc.sync.dma_start(rhi[:], rhi_t[:])
        deg = pool.tile([128, TPCg], F32)
        nc.vector.tensor_sub(deg[:], rhi[:], rlo[:])
        degm = pool.tile([128, TPCg], F32)
        nc.vector.tensor_scalar_max(degm[:], deg[:], 1.0)
        sq = pool.tile([128, TPCg], F32)
        nc.scalar.sqrt(sq[:], degm[:])
        rs = pool.tile([128, TPCg], F32)
        nc.vector.reciprocal(rs[:], sq[:])
        mask = pool.tile([128, TPCg], F32)
        nc.vector.tensor_scalar(mask[:], deg[:], 0.0, None, mybir.AluOpType.is_gt)
        dinv = pool.tile([128, TPCg], F32)
        nc.vector.tensor_mul(dinv[:], rs[:], mask[:])
        nc.sync.dma_start(dinv_t[:], dinv[:])
    nc.compile()
    return nc


# ---------------------------------------------------------------------------
# Program 2: the full GCN
# ---------------------------------------------------------------------------

def build_main_program(c):
    M, TPC, capH, LH, LHP, half, npc, npad = (c["M"], c["TPC"], c["capH"], c["LH"],
                                              c["LHP"], c["half"], c["npc"], c["npad"])
    C, HID, OUT, GCALL, EPS, N = c["C"], c["HID"], c["OUT"], c["GCALL"], c["EPS"], c["N"]
    SB, NQ = c["SB"], c["NQ"]
    CPH = LHP // 128          # chunks per half (incl pad chunks)
    RCH = LH // 128           # real chunks per half
    CPT = capH // 128         # chunks per (tile, half) group
    NCALL = LHP // GCALL
    CPC = GCALL // 128        # chunks per gather call
    NSB = (2 * RCH + SB - 1) // SB   # S batches

    nc = bacc.Bacc("TRN2", target_bir_lowering=False, debug=False, num_devices=M,
                   num_swdge_queues=NQ, dynamic_dma_scratch_size=c.get("SCRATCH", 16384))
    xp_t = nc.dram_tensor("x_pad", [npad, C], BF16, kind="ExternalInput")
    idxA_t = nc.dram_tensor("idxA", [128, LHP // 16], I16, kind="ExternalInput")
    idxB_t = nc.dram_tensor("idxB", [128, LHP // 16], I16, kind="ExternalInput")
    S_t = nc.dram_tensor("S", [128, 2 * RCH * 128], BF16, kind="ExternalInput")
    dloc_t = nc.dram_tensor("dloc", [128, TPC], F32, kind="ExternalInput")
    valid_t = nc.dram_tensor("valid", [128, TPC], F32, kind="ExternalInput")
    W1_t = nc.dram_tensor("W1", [C, HID], F32, kind="ExternalInput")
    Wf_t = nc.dram_tensor("Wf", [HID, C], F32, kind="ExternalInput")
    Wo_t = nc.dram_tensor("Wo", [HID, OUT], F32, kind="ExternalInput")
    gamma_t = nc.dram_tensor("gamma", [1, HID], F32, kind="ExternalInput")
    beta_t = nc.dram_tensor("beta", [1, HID], F32, kind="ExternalInput")
    bfb_t = nc.dram_tensor("bfb", [128, C], F32, kind="ExternalInput")
    bob_t = nc.dram_tensor("bob", [128, OUT], F32, kind="ExternalInput")
    feat_t = nc.dram_tensor("features", [npc, C], F32, kind="ExternalOutput")
    logi_t = nc.dram_tensor("logits", [npc, OUT], F32, kind="ExternalOutput")

    groups = [list(range(M))]

    with tile.TileContext(nc) as tc, ExitStack() as ctx:
        const = ctx.enter_context(tc.tile_pool(name="const", bufs=1))
        tab = ctx.enter_context(tc.tile_pool(name="tab", bufs=1))
        zpool = ctx.enter_context(tc.tile_pool(name="zpool", bufs=1))
        accp = ctx.enter_context(tc.tile_pool(name="accp", bufs=1))
        mpool = ctx.enter_context(tc.tile_pool(name="mpool", bufs=c.get("MBUFS", 6)))
        spool = ctx.enter_context(tc.tile_pool(name="spool", bufs=3))
        tmpp = ctx.enter_context(tc.tile_pool(name="tmpp", bufs=4))
        psA = ctx.enter_context(tc.tile_pool(name="psA", bufs=2, space="PSUM"))
        psY = ctx.enter_context(tc.tile_pool(name="psY", bufs=2, space="PSUM"))
        psS = ctx.enter_context(tc.tile_pool(name="psS", bufs=1, space="PSUM"))
        dram = ctx.enter_context(tc.tile_pool(name="dram", bufs=1, space="DRAM"))

        ones1 = const.tile([1, 128], F32)
        nc.gpsimd.memset(ones1[:], 1.0)
        W1_sb = const.tile([C, HID], F32)
        nc.sync.dma_start(W1_sb[:], W1_t[:])
        Wf_sb = const.tile([HID, C], F32)
        nc.sync.dma_start(Wf_sb[:], Wf_t[:])
        Wo_sb = const.tile([HID, OUT], F32)
        nc.sync.dma_start(Wo_sb[:], Wo_t[:])
        gamma_sb = const.tile([1, HID], F32)
        nc.sync.dma_start(gamma_sb[:], gamma_t[:])
        beta_sb = const.tile([1, HID], F32)
        nc.sync.dma_start(beta_sb[:], beta_t[:])
        bfb_sb = const.tile([128, C], F32)
        nc.sync.dma_start(bfb_sb[:], bfb_t[:])
        bob_sb = const.tile([128, OUT], F32)
        nc.sync.dma_start(bob_sb[:], bob_t[:])
        dloc_sb = const.tile([128, TPC], F32)
        nc.sync.dma_start(dloc_sb[:], dloc_t[:])
        valid_sb = const.tile([128, TPC], F32)
        nc.sync.dma_start(valid_sb[:], valid_t[:])

        idx_sb = []
        for nm, t in (("idxA", idxA_t), ("idxB", idxB_t)):
            s = tab.tile([128, LHP // 16], I16, name=nm + "_sb", tag=nm)
            nc.sync.dma_start(s[:], t[:])
            idx_sb.append(s)

        h_loc = dram.tile([npc, C], BF16)
        h_full = dram.tile([npad, C], BF16, addr_space="Shared")
        st_in = dram.tile([2, HID], F32)
        st_out = dram.tile([2, HID], F32, addr_space="Shared")

        qctr = [0]

        def sparse_pass(src_dram, acc_prefix):
            """Gather + one-hot segment-sum both halves; returns dict t -> acc
            SBUF tile [C, 128] f32 (channels on partitions)."""
            accs = {}
            qi = 0            # real-chunk counter (indexes S batches)
            sb_tile = [None]
            for h in (0, 1):
                base = src_dram[h * half:(h + 1) * half, :]
                ps = None
                for k in range(NCALL):
                    msg = mpool.tile([128, CPC, C], BF16, name=f"msg_{acc_prefix}",
                                     tag="msg")
                    nc.gpsimd.dma_gather(msg[:], base,
                                         idx_sb[h][:, k * (GCALL // 16):(k + 1) * (GCALL // 16)],
                                         GCALL, GCALL, C, queue_num=qctr[0] % NQ)
                    qctr[0] += 1
                    for j in range(CPC):
                        qq = k * CPC + j
                        if qq >= RCH:
                            continue
                        if qi % SB == 0:
                            nb = min(SB, 2 * RCH - qi)
                            st = spool.tile([128, nb, 128], BF16,
                                            name=f"Sb_{acc_prefix}", tag="Sb")
                            nc.sync.dma_start(st[:], S_t[:, qi * 128:(qi + nb) * 128])
                            sb_tile[0] = st
                        t = qq // CPT
                        jj = qq % CPT
                        if jj == 0:
                            ps = psA.tile([C, 128], F32, name=f"ps_{acc_prefix}",
                                          tag="psacc")
                        nc.tensor.matmul(ps[:], lhsT=msg[:, j, :],
                                         rhs=sb_tile[0][:, qi % SB, :],
                                         start=(jj == 0), stop=(jj == CPT - 1))
                        qi += 1
                        if jj == CPT - 1:
                            if h == 0:
                                a = accp.tile([C, 128], F32, name=f"{acc_prefix}_{t}",
                                              tag=f"acc_{t}")
                                nc.vector.tensor_copy(a[:], ps[:])
                                accs[t] = a
                            else:
                                nc.vector.tensor_add(accs[t][:], accs[t][:], ps[:])
            return accs

        # ================= layer 1 =================
        accs1 = sparse_pass(xp_t, "l1")
        st_sum = psS.tile([1, HID], F32, tag="ssum")
        st_sq = psS.tile([1, HID], F32, tag="ssq")
        z_tiles = []
        for t in range(TPC):
            y = psY.tile([128, HID], F32, name="y1", tag="y")
            nc.tensor.matmul(y[:], lhsT=accs1[t][:], rhs=W1_sb[:], start=True, stop=True)
            z = zpool.tile([128, HID], F32, name=f"z_{t}", tag=f"z_{t}")
            nc.scalar.activation(z[:], y[:], mybir.ActivationFunctionType.Copy,
                                 scale=dloc_sb[:, t:t + 1])
            z_tiles.append(z)
            sq = tmpp.tile([128, HID], F32, name="sqz", tag="sqz")
            nc.scalar.square(sq[:], z[:])
            nc.tensor.matmul(st_sum[:], lhsT=valid_sb[:, t:t + 1], rhs=z[:],
                             start=(t == 0), stop=(t == TPC - 1))
            nc.tensor.matmul(st_sq[:], lhsT=valid_sb[:, t:t + 1], rhs=sq[:],
                             start=(t == 0), stop=(t == TPC - 1))

        # ================= BN stats -> coefficients =================
        st_sb0 = tmpp.tile([1, HID], F32, tag="stats0", bufs=1)
        nc.vector.tensor_copy(st_sb0[:], st_sum[:])
        st_sb1 = tmpp.tile([1, HID], F32, tag="stats1", bufs=1)
        nc.vector.tensor_copy(st_sb1[:], st_sq[:])
        nc.sync.dma_start(st_in[0:1, :], st_sb0[:])
        nc.sync.dma_start(st_in[1:2, :], st_sb1[:])
        nc.gpsimd.collective_compute("AllReduce", mybir.AluOpType.add,
                                     replica_groups=groups,
                                     ins=[st_in[:]], outs=[st_out[:]])
        st20 = tmpp.tile([1, HID], F32, tag="stats20", bufs=1)
        nc.sync.dma_start(st20[:], st_out[0:1, :])
        st21 = tmpp.tile([1, HID], F32, tag="stats21", bufs=1)
        nc.sync.dma_start(st21[:], st_out[1:2, :])
        invN = 1.0 / float(N)
        mean = tmpp.tile([1, HID], F32, tag="bnrow", bufs=8)
        nc.vector.tensor_scalar(mean[:], st20[:], invN, None, mybir.AluOpType.mult)
        ex2 = tmpp.tile([1, HID], F32, tag="bnrow", bufs=8)
        nc.vector.tensor_scalar(ex2[:], st21[:], invN, None, mybir.AluOpType.mult)
        msq = tmpp.tile([1, HID], F32, tag="bnrow", bufs=8)
        nc.scalar.square(msq[:], mean[:])
        var = tmpp.tile([1, HID], F32, tag="bnrow", bufs=8)
        nc.vector.tensor_sub(var[:], ex2[:], msq[:])
        varp = tmpp.tile([1, HID], F32, tag="bnrow", bufs=8)
        nc.vector.tensor_scalar_add(varp[:], var[:], float(EPS))
        std = tmpp.tile([1, HID], F32, tag="bnrow", bufs=8)
        nc.scalar.sqrt(std[:], varp[:])
        istd = tmpp.tile([1, HID], F32, tag="bnrow", bufs=8)
        nc.vector.reciprocal(istd[:], std[:])
        scal = tmpp.tile([1, HID], F32, tag="bnrow", bufs=8)
        nc.vector.tensor_mul(scal[:], gamma_sb[:], istd[:])
        mscal = tmpp.tile([1, HID], F32, tag="bnrow", bufs=8)
        nc.vector.tensor_mul(mscal[:], mean[:], scal[:])
        shif = tmpp.tile([1, HID], F32, tag="bnrow", bufs=8)
        nc.vector.tensor_sub(shif[:], beta_sb[:], mscal[:])
        scale_b = const.tile([128, HID], F32)
        shift_b = const.tile([128, HID], F32)
        for row, dst_tile in ((scal, scale_b), (shif, shift_b)):
            pb = psY.tile([128, HID], F32, name="pb", tag="y")
            nc.tensor.matmul(pb[:], lhsT=ones1[:], rhs=row[:], start=True, stop=True)
            nc.vector.tensor_copy(dst_tile[:], pb[:])

        # ========== BN apply + ReLU + dinv scale (hs, bf16) + AllGather ==========
        for t in range(TPC):
            t1 = tmpp.tile([128, HID], F32, name="bn1", tag="bn1")
            nc.vector.tensor_mul(t1[:], z_tiles[t][:], scale_b[:])
            t2 = tmpp.tile([128, HID], F32, name="bn2", tag="bn2")
            nc.vector.tensor_add(t2[:], t1[:], shift_b[:])
            ht = tmpp.tile([128, HID], BF16, name="ht", tag="ht")
            # relu(x)*d == relu(x*d) for d >= 0; dloc >= 0
            nc.scalar.activation(ht[:], t2[:], mybir.ActivationFunctionType.Relu,
                                 scale=dloc_sb[:, t:t + 1])
            nc.sync.dma_start(h_loc[t * 128:(t + 1) * 128, :], ht[:])
        nc.gpsimd.collective_compute("AllGather", mybir.AluOpType.bypass,
                                     replica_groups=groups,
                                     ins=[h_loc[:]], outs=[h_full[:]])

        # ================= layer 2 =================
        accs2 = sparse_pass(h_full, "l2")
        for t in range(TPC):
            yf = psY.tile([128, C], F32, name="yf", tag="y")
            nc.tensor.matmul(yf[:], lhsT=accs2[t][:], rhs=Wf_sb[:], start=True, stop=True)
            ft = tmpp.tile([128, C], F32, name="ft", tag="ft")
            nc.scalar.activation(ft[:], yf[:], mybir.ActivationFunctionType.Copy,
                                 scale=dloc_sb[:, t:t + 1])
            ft2 = tmpp.tile([128, C], F32, name="ft2", tag="ft2")
            nc.vector.tensor_add(ft2[:], ft[:], bfb_sb[:])
            nc.sync.dma_start(feat_t[t * 128:(t + 1) * 128, :], ft2[:])
            yl = psY.tile([128, OUT], F32, name="yl", tag="yl")
            nc.tensor.matmul(yl[:], lhsT=accs2[t][:], rhs=Wo_sb[:], start=True, stop=True)
            lt = tmpp.tile([128, OUT], F32, name="lt", tag="lt")
            nc.scalar.activation(lt[:], yl[:], mybir.ActivationFunctionType.Copy,
                                 scale=dloc_sb[:, t:t + 1])
            lt2 = tmpp.tile([128, OUT], F32, name="lt2", tag="lt2")
            nc.vector.tensor_add(lt2[:], lt[:], bob_sb[:])
            nc.sync.dma_start(logi_t[t * 128:(t + 1) * 128, :], lt2[:])

    nc.compile()
    return nc


# ---------------------------------------------------------------------------
# kernel()
# ---------------------------------------------------------------------------

def run_gcn(x, edge_index, W1, b1, gamma, beta, Wf, bf, Wo, bo, cfg=None,
            runner=None, trace=False):
    """Full pipeline. runner: callable(nc, in_maps, core_ids) -> results list
    (defaults to hardware via run_bass_kernel_spmd)."""
    global LAST_EXEC_NS, LAST_TRACE, LAST_PROFILE_JSON
    cfg = dict(CFG if cfg is None else cfg)
    c, per_core = prep_graph(edge_index, cfg)
    M, TPC, npc, nper, N = c["M"], c["TPC"], c["npc"], c["nper"], c["N"]

    x = np.asarray(x, np.float32)
    W1 = np.asarray(W1, np.float32)
    gamma = np.asarray(gamma, np.float32).reshape(1, -1)
    beta = np.asarray(beta, np.float32).reshape(1, -1)
    Wf = np.asarray(Wf, np.float32)
    bf = np.asarray(bf, np.float32)
    Wo = np.asarray(Wo, np.float32)
    bo = np.asarray(bo, np.float32)

    # --- program 1: dinv on device (single core) ---
    rlo = np.concatenate([pc["rlo"].reshape(TPC, 128).T for pc in per_core], axis=1)
    rhi = np.concatenate([pc["rhi"].reshape(TPC, 128).T for pc in per_core], axis=1)
    nc1 = build_dinv_program(c)
    in1 = [dict(rlo=np.ascontiguousarray(rlo), rhi=np.ascontiguousarray(rhi))]
    if runner is None:
        res1 = run_bass_kernel_spmd(nc1, in1, [0]).results
    else:
        res1 = runner(nc1, in1, [0])
    dinv_pad = res1[0]["dinv"].T.reshape(-1)         # padded-global order

    # --- host table prep (device-computed dinv, host layout) ---
    tables = prep_tables(c, per_core, dinv_pad)
    xp = pad_x(x, c)
    xs = (xp * dinv_pad[:, None]).astype(NP_BF16)    # src-side scale folded in
    bfb = np.tile(bf.reshape(1, -1), (128, 1)).astype(np.float32)
    bob = np.tile(bo.reshape(1, -1), (128, 1)).astype(np.float32)

    # --- program 2 ---
    nc2 = build_main_program(c)
    in_maps = []
    for ci in range(M):
        tb = tables[ci]
        in_maps.append(dict(x_pad=xs, idxA=tb["idxA"], idxB=tb["idxB"], S=tb["S"],
                            dloc=tb["dloc"], valid=tb["valid"], W1=W1, Wf=Wf, Wo=Wo,
                            gamma=gamma, beta=beta, bfb=bfb, bob=bob))
    core_ids = list(range(M))
    if runner is None:
        if trace and not _ensure_ntff_hook():
            trace = False
        if trace:
            import concourse.bass_utils as _bu
            _bu.upload_artifacts = lambda d: str(d)
        try:
            r = run_bass_kernel_spmd(nc2, in_maps, core_ids, trace=trace)
        except Exception:
            if not trace:
                raise
            import traceback
            traceback.print_exc()
            print("trace path failed; re-running without trace", file=sys.stderr)
            r = run_bass_kernel_spmd(nc2, in_maps, core_ids, trace=False)
        if getattr(r, "exec_time_ns", None):
            LAST_EXEC_NS = r.exec_time_ns
        if getattr(r, "instructions_and_trace", None):
            LAST_TRACE = r.instructions_and_trace[1]
        if getattr(r, "profile_json", None):
            LAST_PROFILE_JSON = r.profile_json
        res2 = r.results
    else:
        res2 = runner(nc2, in_maps, core_ids)

    features = np.zeros((N, c["C"]), np.float32)
    logits = np.zeros((N, c["OUT"]), np.float32)
    for ci in range(M):
        nv = min(nper, N - ci * nper)
        features[ci * nper:ci * nper + nv] = res2[ci]["features"][:nv]
        logits[ci * nper:ci * nper + nv] = res2[ci]["logits"][:nv]
    return features, logits


def kernel(**inputs):
    feats, logits = run_gcn(**inputs)
    return feats, logits


if __name__ == "__main__":
    pass
